# revision 1
# baseline (speedup 1.0000x reference)
"""FP8GroupedExperts Trainium2 kernel (expert-parallel over 8 NeuronCores).

Math per expert e (tokens pre-sorted by expert, n_e tokens each):
    h   = silu(x_e @ w1[e].T) * (x_e @ w3[e].T)      # (n_e, HID) SwiGLU
    out = h @ w2[e].T                                 # (n_e, DIM)

Sharding: one expert per core (E == n_cores == 8). Each core's segment is
padded to T tokens; zero rows produce zero outputs (silu(0)*0 == 0) and are
stripped on the host after the gather.

On-chip layout (zero on-chip transposes):
  phase 1 computes h^T tiles (HID on partitions, tokens on free dim):
      y1T = W1 @ x^T  via  matmul(lhsT=W1T chunk, rhs=xT chunk)
      hT  = silu(y1T) * y3T
  phase 2 computes out = h @ W2^T with the HID contraction on partitions:
      out tile = matmul(lhsT=hT chunk, rhs=W2T chunk)
All operands are host-pre-transposed so every DMA is dense.
"""

import sys

for _p in ("/opt/trn_rl_repo",):
    if _p not in sys.path:
        sys.path.append(_p)

import numpy as np
import ml_dtypes

import concourse.bacc as bacc
import concourse.mybir as mybir
import concourse.tile as tile
from concourse.bass import ts
from concourse.bass_utils import run_bass_kernel_spmd

E = 8
DIM = 2048
HID = 5632
T = 2048          # padded tokens per expert (= TOT // E)
P = 128

T_TILE = 512
NT = T // T_TILE            # 4 token tiles
KSUB = DIM // P             # 16 contraction subtiles for phase 1
HSUB = HID // P             # 44 h subtiles (phase-1 outputs / phase-2 contraction)
HG = 4                      # h-subtiles per weight-group DMA
NHG = HSUB // HG            # 11 weight groups
ND = DIM // 512             # 4 output column chunks (phase 2)
NTS = T_TILE // P           # 4 token partition-subtiles (phase 2)

BF16 = mybir.dt.bfloat16
F32 = mybir.dt.float32

_BUILD_CACHE = {}


def _build():
    """Build the per-core Bass module (same NEFF for all 8 cores)."""
    nc = bacc.Bacc(None, target_bir_lowering=False)

    xt = nc.dram_tensor("xt", [DIM, T], BF16, kind="ExternalInput")
    w1t = nc.dram_tensor("w1t", [DIM, HID], BF16, kind="ExternalInput")
    w3t = nc.dram_tensor("w3t", [DIM, HID], BF16, kind="ExternalInput")
    w2t = nc.dram_tensor("w2t", [HID, DIM], BF16, kind="ExternalInput")
    out = nc.dram_tensor("out", [T, DIM], F32, kind="ExternalOutput")

    xt_r = xt.rearrange("(ko p) t -> p ko t", p=P)      # [128, 16, 2048]
    w1_r = w1t.rearrange("(ko p) h -> p ko h", p=P)     # [128, 16, 5632]
    w3_r = w3t.rearrange("(ko p) h -> p ko h", p=P)

    w2_r = w2t.rearrange("(hh p) d -> p hh d", p=P)     # [128, 44, 2048]

    with tile.TileContext(nc) as tc:
        with (
            tc.tile_pool(name="xp", bufs=2) as xp,
            tc.tile_pool(name="wp", bufs=2) as wp,
            tc.tile_pool(name="hp", bufs=1) as hp,
            tc.tile_pool(name="tp", bufs=3) as tp,
            tc.tile_pool(name="w2p", bufs=4) as w2p,
            tc.tile_pool(name="op", bufs=4) as op,
            tc.tile_pool(name="ps1", bufs=1, space="PSUM") as ps1,
            tc.tile_pool(name="ps2", bufs=6, space="PSUM") as ps2,
        ):
            def load_x(it, split=1):
                # split>1 issues per-k-chunk DMAs so the first matmuls (which
                # only read low k subtiles) can start before the full tile lands
                t0 = it * T_TILE
                xtt = xp.tile([P, KSUB, T_TILE], BF16, tag="xtt", name=f"xtt_{it}")
                kc = KSUB // split
                for s in range(split):
                    nc.sync.dma_start(
                        xtt[:, s * kc:(s + 1) * kc, :],
                        xt_r[:, s * kc:(s + 1) * kc, t0:t0 + T_TILE],
                    )
                return xtt

            def load_wg(it, hg, split=1):
                h0 = hg * HG * P
                w1g = wp.tile(
                    [P, KSUB, HG * P], BF16, tag="w1g", name=f"w1g_{it}_{hg}"
                )
                w3g = wp.tile(
                    [P, KSUB, HG * P], BF16, tag="w3g", name=f"w3g_{it}_{hg}"
                )
                kc = KSUB // split
                for s in range(split):
                    ks = slice(s * kc, (s + 1) * kc)
                    nc.sync.dma_start(w1g[:, ks, :], w1_r[:, ks, h0:h0 + HG * P])
                    nc.sync.dma_start(w3g[:, ks, :], w3_r[:, ks, h0:h0 + HG * P])
                return w1g, w3g

            pending = {}

            # HAM pre-warm: the PE clock gate defaults to 1.2GHz and opens to
            # 2.4GHz only after ~3.4us of sustained activity. Run zero-operand
            # matmuls into a scratch PSUM bank while the first input DMAs are
            # in flight, so real matmuls start at full clock. The scratch tile
            # shares the py1 slot; PE program order keeps the reuse safe.
            wz = tp.tile([P, 512], BF16, tag="wz", bufs=1)
            nc.gpsimd.memset(wz[:], 0.0)
            wps = ps1.tile([P, T_TILE], F32, tag="py1", name="warm_ps")
            for _ in range(24):
                nc.tensor.matmul(wps[:], wz[:, 0:P], wz[:], start=True, stop=True)

            # iter-0 startup: interleave small per-k chunks of x/w1/w3 so the
            # first matmul's operands land after one chunk of each instead of
            # after the whole x tile (DMA issue on the sync queue serializes)
            xtt0 = xp.tile([P, KSUB, T_TILE], BF16, tag="xtt", name="xtt_0")
            w1g0 = wp.tile([P, KSUB, HG * P], BF16, tag="w1g", name="w1g_0_0")
            w3g0 = wp.tile([P, KSUB, HG * P], BF16, tag="w3g", name="w3g_0_0")
            for s in range(4):
                ks = slice(s * (KSUB // 4), (s + 1) * (KSUB // 4))
                nc.sync.dma_start(xtt0[:, ks, :], xt_r[:, ks, 0:T_TILE])
                nc.sync.dma_start(w1g0[:, ks, :], w1_r[:, ks, 0:HG * P])
                nc.sync.dma_start(w3g0[:, ks, :], w3_r[:, ks, 0:HG * P])
            pending[("x", 0)] = xtt0
            pending[("w", 0, 0)] = (w1g0, w3g0)

            for it in range(NT):
                t0 = it * T_TILE
                xtt = pending.pop(("x", it), None)
                if xtt is None:
                    xtt = load_x(it)
                hsb = hp.tile([P, HSUB, T_TILE], BF16, tag="hsb")

                # ---- phase 1: hT = silu(W1 xT) * (W3 xT), HID on partitions
                for hg in range(NHG):
                    wg = pending.pop(("w", it, hg), None)
                    if wg is None:
                        wg = load_wg(it, hg)
                    w1g, w3g = wg
                    for hh in range(HG):
                        h = hg * HG + hh
                        py1 = ps1.tile([P, T_TILE], F32, tag="py1")
                        for k in range(KSUB):
                            nc.tensor.matmul(
                                py1[:], w1g[:, k, ts(hh, P)], xtt[:, k, :],
                                start=(k == 0), stop=(k == KSUB - 1),
                            )
                        py3 = ps1.tile([P, T_TILE], F32, tag="py3")
                        for k in range(KSUB):
                            nc.tensor.matmul(
                                py3[:], w3g[:, k, ts(hh, P)], xtt[:, k, :],
                                start=(k == 0), stop=(k == KSUB - 1),
                            )
                        smp = tp.tile([P, T_TILE], F32, tag="smp")
                        nc.scalar.activation(
                            smp[:], py1[:], mybir.ActivationFunctionType.Silu
                        )
                        nc.vector.tensor_tensor(
                            hsb[:, h, :], smp[:], py3[:], mybir.AluOpType.mult
                        )

                # head-start phase 2's first two w2 tiles, then prefetch the
                # next iter's activations + first weight group: the head tiles
                # aren't queued behind the 6MB prefetch, and the prefetch still
                # issues before the rest of the w2 stream
                W2B = 4  # h-subtiles per w2 DMA
                w2_head = []
                for hb in range(2):
                    w2g = w2p.tile([P, W2B, 512], BF16, tag="w2g",
                                   name=f"w2head_{it}_{hb}")
                    nc.sync.dma_start(
                        w2g[:], w2_r[:, hb * W2B:(hb + 1) * W2B, 0:512]
                    )
                    w2_head.append(w2g)
                if it + 1 < NT:
                    pending[("x", it + 1)] = load_x(it + 1)
                    pending[("w", it + 1, 0)] = load_wg(it + 1, 0)

                # ---- phase 2: out tile = hT.T @ W2T, contraction over HID
                for d in range(ND):
                    pos = [
                        ps2.tile([P, 512], F32, tag="po", name=f"po_{i}")
                        for i in range(NTS)
                    ]
                    for hb in range(HSUB // W2B):
                        if d == 0 and hb < 2:
                            w2g = w2_head[hb]
                        else:
                            w2g = w2p.tile([P, W2B, 512], BF16, tag="w2g")
                            nc.sync.dma_start(
                                w2g[:],
                                w2_r[:, hb * W2B:(hb + 1) * W2B,
                                     d * 512:(d + 1) * 512],
                            )
                        for hh in range(W2B):
                            h = hb * W2B + hh
                            for i in range(NTS):
                                nc.tensor.matmul(
                                    pos[i][:], hsb[:, h, ts(i, P)], w2g[:, hh, :],
                                    start=(h == 0), stop=(h == HSUB - 1),
                                )
                    for i in range(NTS):
                        osb = op.tile([P, 512], F32, tag="osb")
                        nc.vector.tensor_copy(osb[:], pos[i][:])
                        nc.sync.dma_start(
                            out[t0 + i * P:t0 + (i + 1) * P, d * 512:(d + 1) * 512],
                            osb[:],
                        )

    nc.compile()
    return nc


def _get_nc():
    if "nc" not in _BUILD_CACHE:
        _BUILD_CACHE["nc"] = _build()
    return _BUILD_CACHE["nc"]


def _prep_inputs(x, num_tokens_per_expert, w1, w2, w3):
    """Host-side shard + layout prep: per-expert transposed bf16 operands."""
    x = np.asarray(x, dtype=np.float32)
    w1 = np.asarray(w1)
    w2 = np.asarray(w2)
    w3 = np.asarray(w3)
    counts = np.asarray(num_tokens_per_expert).astype(np.int64)
    offs = np.concatenate([[0], np.cumsum(counts)])

    in_maps = []
    for e in range(E):
        n_e = int(counts[e])
        if n_e > T:
            raise ValueError(f"expert {e} has {n_e} tokens > padded capacity {T}")
        xe = x[offs[e]:offs[e] + n_e]
        if n_e < T:
            xe = np.concatenate(
                [xe, np.zeros((T - n_e, DIM), dtype=np.float32)], axis=0
            )
        in_maps.append({
            "xt": np.ascontiguousarray(xe.T).astype(ml_dtypes.bfloat16),
            "w1t": np.ascontiguousarray(np.asarray(w1[e]).T).astype(ml_dtypes.bfloat16),
            "w3t": np.ascontiguousarray(np.asarray(w3[e]).T).astype(ml_dtypes.bfloat16),
            "w2t": np.ascontiguousarray(np.asarray(w2[e]).T).astype(ml_dtypes.bfloat16),
        })
    return in_maps, counts


def _run(inputs, **run_kwargs):
    in_maps, counts = _prep_inputs(
        inputs["x"], inputs["num_tokens_per_expert"],
        inputs["w1"], inputs["w2"], inputs["w3"],
    )
    nc = _get_nc()
    res = run_bass_kernel_spmd(nc, in_maps, core_ids=list(range(E)), **run_kwargs)
    pieces = [res.results[e]["out"][: int(counts[e])] for e in range(E)]
    full = np.concatenate(pieces, axis=0).astype(np.float32)
    return full, res


def kernel(**inputs):
    out, _ = _run(inputs)
    return out


if __name__ == "__main__":
    # Tiny self-check with random data (not the reference inputs).
    rng = np.random.default_rng(0)
    ins = {
        "x": rng.standard_normal((E * T, DIM), dtype=np.float32),
        "num_tokens_per_expert": np.full((E,), T, dtype=np.int64),
        "w1": rng.standard_normal((E, HID, DIM), dtype=np.float32) * 0.02,
        "w2": rng.standard_normal((E, DIM, HID), dtype=np.float32) * 0.02,
        "w3": rng.standard_normal((E, HID, DIM), dtype=np.float32) * 0.02,
    }
    got = kernel(**ins)
    print("out shape:", got.shape, got.dtype)



# revision 7
# speedup vs baseline: 1.2406x; 1.2406x over previous
"""FP8GroupedExperts Trainium2 kernel (expert-parallel over 8 NeuronCores).

Math per expert e (tokens pre-sorted by expert, n_e tokens each):
    h   = silu(x_e @ w1[e].T) * (x_e @ w3[e].T)      # (n_e, HID) SwiGLU
    out = h @ w2[e].T                                 # (n_e, DIM)

Sharding: one expert per core (E == n_cores == 8). Each core's segment is
padded to T tokens; zero rows produce zero outputs (silu(0)*0 == 0) and are
stripped on the host after the gather.

On-chip layout (zero on-chip transposes):
  phase 1 computes h^T tiles (HID on partitions, tokens on free dim):
      y1T = W1 @ x^T  via  matmul(lhsT=W1T chunk, rhs=xT chunk)
      hT  = silu(y1T) * y3T
  phase 2 computes out = h @ W2^T with the HID contraction on partitions:
      out tile = matmul(lhsT=hT chunk, rhs=W2T chunk)
All operands are host-pre-transposed so every DMA is dense.
"""

import sys

for _p in ("/opt/trn_rl_repo",):
    if _p not in sys.path:
        sys.path.append(_p)

import numpy as np
import ml_dtypes

import concourse.bacc as bacc
import concourse.mybir as mybir
import concourse.tile as tile
from concourse.bass import ts
from concourse.bass_utils import run_bass_kernel_spmd

E = 8
DIM = 2048
HID = 5632
T = 2048          # padded tokens per expert (= TOT // E)
P = 128

T_TILE = 512
NT = T // T_TILE            # 4 token tiles
KSUB = DIM // P             # 16 contraction subtiles for phase 1
HSUB = HID // P             # 44 h subtiles (phase-1 outputs / phase-2 contraction)
HG = 4                      # h-subtiles per weight-group DMA
NHG = HSUB // HG            # 11 weight groups
ND = DIM // 512             # 4 output column chunks (phase 2)
NTS = T_TILE // P           # 4 token partition-subtiles (phase 2)

# Partial-fp8 phase 2: the first N8H h-subtiles of the HID contraction run as
# e4m3 DoubleRow pairs (2 subtiles per matmul at ~2x rate). h is quantized at
# scale 1 (values ~N(0,0.5), well inside e4m3 range); w2 is pre-scaled by 2^10
# on the host for BOTH the fp8 and bf16 parts (a lossless exponent shift for
# bf16), so the PSUM accumulates out*2^10 uniformly and one scaled copy
# descales it. Error budget: full-fp8 phase 2 measures 3.75% rel err vs the
# fp32 oracle; N8H/44 of the contraction in fp8 gives 3.75%*sqrt(N8H/44),
# +0.4% bf16 baseline in quadrature -> ~1.8% at N8H=10 vs the 2e-2 gate.
N8H = 10                    # fp8 h-subtiles (must be even)
N16H = HSUB - N8H           # remaining bf16 h-subtiles
W2SCALE = 1024.0            # host-side w2 pre-scale (power of 2)

BF16 = mybir.dt.bfloat16
F8E4 = mybir.dt.float8e4
F32 = mybir.dt.float32

_BUILD_CACHE = {}


def _build():
    """Build the per-core Bass module (same NEFF for all 8 cores)."""
    nc = bacc.Bacc(None, target_bir_lowering=False)

    xt = nc.dram_tensor("xt", [DIM, T], BF16, kind="ExternalInput")
    w1t = nc.dram_tensor("w1t", [DIM, HID], BF16, kind="ExternalInput")
    w3t = nc.dram_tensor("w3t", [DIM, HID], BF16, kind="ExternalInput")
    w2t8 = nc.dram_tensor("w2t8", [N8H * P, DIM], F8E4, kind="ExternalInput")
    w2t16 = nc.dram_tensor("w2t16", [N16H * P, DIM], BF16, kind="ExternalInput")
    out = nc.dram_tensor("out", [T, DIM], F32, kind="ExternalOutput")

    xt_r = xt.rearrange("(ko p) t -> p ko t", p=P)      # [128, 16, 2048]
    w1_r = w1t.rearrange("(ko p) h -> p ko h", p=P)     # [128, 16, 5632]
    w3_r = w3t.rearrange("(ko p) h -> p ko h", p=P)

    w2_r8 = w2t8.rearrange("(hh p) d -> p hh d", p=P)    # [128, 10, 2048]
    w2_r16 = w2t16.rearrange("(hh p) d -> p hh d", p=P)  # [128, 34, 2048]

    with tile.TileContext(nc) as tc:
        with (
            tc.tile_pool(name="xp", bufs=2) as xp,
            tc.tile_pool(name="wp", bufs=2) as wp,
            tc.tile_pool(name="hp", bufs=1) as hp,
            tc.tile_pool(name="tp", bufs=3) as tp,
            tc.tile_pool(name="w2p", bufs=4) as w2p,
            tc.tile_pool(name="op", bufs=4) as op,
            tc.tile_pool(name="ps1", bufs=1, space="PSUM") as ps1,
            tc.tile_pool(name="ps2", bufs=6, space="PSUM") as ps2,
        ):
            def load_x(it, split=1):
                # split>1 issues per-k-chunk DMAs so the first matmuls (which
                # only read low k subtiles) can start before the full tile lands
                t0 = it * T_TILE
                xtt = xp.tile([P, KSUB, T_TILE], BF16, tag="xtt", name=f"xtt_{it}")
                kc = KSUB // split
                for s in range(split):
                    nc.sync.dma_start(
                        xtt[:, s * kc:(s + 1) * kc, :],
                        xt_r[:, s * kc:(s + 1) * kc, t0:t0 + T_TILE],
                    )
                return xtt

            def load_wg(it, hg, split=1):
                h0 = hg * HG * P
                w1g = wp.tile(
                    [P, KSUB, HG * P], BF16, tag="w1g", name=f"w1g_{it}_{hg}"
                )
                w3g = wp.tile(
                    [P, KSUB, HG * P], BF16, tag="w3g", name=f"w3g_{it}_{hg}"
                )
                kc = KSUB // split
                for s in range(split):
                    ks = slice(s * kc, (s + 1) * kc)
                    nc.sync.dma_start(w1g[:, ks, :], w1_r[:, ks, h0:h0 + HG * P])
                    nc.sync.dma_start(w3g[:, ks, :], w3_r[:, ks, h0:h0 + HG * P])
                return w1g, w3g

            pending = {}

            # HAM pre-warm: the PE clock gate defaults to 1.2GHz and opens to
            # 2.4GHz only after ~3.4us of sustained activity. Run zero-operand
            # matmuls into a scratch PSUM bank while the first input DMAs are
            # in flight, so real matmuls start at full clock. The scratch tile
            # shares the py1 slot; PE program order keeps the reuse safe.
            wz = tp.tile([P, 512], BF16, tag="wz", bufs=1)
            nc.gpsimd.memset(wz[:], 0.0)
            wps = ps1.tile([P, T_TILE], F32, tag="py1", name="warm_ps")
            for _ in range(24):
                nc.tensor.matmul(wps[:], wz[:, 0:P], wz[:], start=True, stop=True)

            # iter-0 startup: interleave small per-k chunks of x/w1/w3 so the
            # first matmul's operands land after one chunk of each instead of
            # after the whole x tile (DMA issue on the sync queue serializes)
            xtt0 = xp.tile([P, KSUB, T_TILE], BF16, tag="xtt", name="xtt_0")
            w1g0 = wp.tile([P, KSUB, HG * P], BF16, tag="w1g", name="w1g_0_0")
            w3g0 = wp.tile([P, KSUB, HG * P], BF16, tag="w3g", name="w3g_0_0")
            for s in range(4):
                ks = slice(s * (KSUB // 4), (s + 1) * (KSUB // 4))
                nc.sync.dma_start(xtt0[:, ks, :], xt_r[:, ks, 0:T_TILE])
                nc.sync.dma_start(w1g0[:, ks, :], w1_r[:, ks, 0:HG * P])
                nc.sync.dma_start(w3g0[:, ks, :], w3_r[:, ks, 0:HG * P])
            pending[("x", 0)] = xtt0
            pending[("w", 0, 0)] = (w1g0, w3g0)

            for it in range(NT):
                t0 = it * T_TILE
                xtt = pending.pop(("x", it), None)
                if xtt is None:
                    xtt = load_x(it)
                hsb8 = hp.tile([P, N8H, T_TILE], F8E4, tag="hsb8")
                hsb16 = hp.tile([P, N16H, T_TILE], BF16, tag="hsb16")

                # ---- phase 1: hT = silu(W1 xT) * (W3 xT), HID on partitions
                for hg in range(NHG):
                    wg = pending.pop(("w", it, hg), None)
                    if wg is None:
                        wg = load_wg(it, hg)
                    w1g, w3g = wg
                    for hh in range(HG):
                        h = hg * HG + hh
                        py1 = ps1.tile([P, T_TILE], F32, tag="py1")
                        for k in range(KSUB):
                            nc.tensor.matmul(
                                py1[:], w1g[:, k, ts(hh, P)], xtt[:, k, :],
                                start=(k == 0), stop=(k == KSUB - 1),
                            )
                        py3 = ps1.tile([P, T_TILE], F32, tag="py3")
                        for k in range(KSUB):
                            nc.tensor.matmul(
                                py3[:], w3g[:, k, ts(hh, P)], xtt[:, k, :],
                                start=(k == 0), stop=(k == KSUB - 1),
                            )
                        smp = tp.tile([P, T_TILE], F32, tag="smp")
                        nc.scalar.activation(
                            smp[:], py1[:], mybir.ActivationFunctionType.Silu
                        )
                        hdst = (
                            hsb8[:, h, :] if h < N8H else hsb16[:, h - N8H, :]
                        )
                        nc.vector.tensor_tensor(
                            hdst, smp[:], py3[:], mybir.AluOpType.mult
                        )

                # head-start phase 2's first w2 tiles (the d=0 fp8 group and
                # first bf16 group), then prefetch the next iter's activations
                # + first weight group: the head tiles aren't queued behind the
                # 6MB prefetch, and the prefetch still issues before the rest
                # of the w2 stream
                W2B = 4  # bf16 h-subtiles per w2 DMA
                w2g8_head = w2p.tile([P, N8H, 512], F8E4, tag="w2g8",
                                     name=f"w2head8_{it}")
                nc.sync.dma_start(w2g8_head[:], w2_r8[:, :, 0:512])
                w2g16_head = w2p.tile([P, W2B, 512], BF16, tag="w2g",
                                      name=f"w2head_{it}")
                nc.sync.dma_start(w2g16_head[:], w2_r16[:, 0:W2B, 0:512])
                if it + 1 < NT:
                    pending[("x", it + 1)] = load_x(it + 1)
                    pending[("w", it + 1, 0)] = load_wg(it + 1, 0)

                # ---- phase 2: out tile = hT.T @ W2T, contraction over HID;
                # fp8 DoubleRow pairs first, then the bf16 remainder
                bf16_bases = list(range(0, N16H, W2B))
                for d in range(ND):
                    pos = [
                        ps2.tile([P, 512], F32, tag="po", name=f"po_{i}")
                        for i in range(NTS)
                    ]
                    if d == 0:
                        w2g8 = w2g8_head
                    else:
                        w2g8 = w2p.tile([P, N8H, 512], F8E4, tag="w2g8")
                        nc.sync.dma_start(
                            w2g8[:], w2_r8[:, :, d * 512:(d + 1) * 512]
                        )
                    for j in range(N8H // 2):
                        for i in range(NTS):
                            nc.tensor.matmul(
                                pos[i][:],
                                hsb8[:, 2 * j:2 * j + 2, ts(i, P)],
                                w2g8[:, 2 * j:2 * j + 2, :],
                                start=(j == 0), stop=False,
                                perf_mode=mybir.MatmulPerfMode.DoubleRow,
                            )
                    for hb in bf16_bases:
                        nb = min(W2B, N16H - hb)
                        if d == 0 and hb == 0:
                            w2g = w2g16_head
                        else:
                            w2g = w2p.tile([P, nb, 512], BF16, tag="w2g")
                            nc.sync.dma_start(
                                w2g[:],
                                w2_r16[:, hb:hb + nb, d * 512:(d + 1) * 512],
                            )
                        for hh in range(nb):
                            h = hb + hh
                            for i in range(NTS):
                                nc.tensor.matmul(
                                    pos[i][:], hsb16[:, h, ts(i, P)],
                                    w2g[:, hh, :],
                                    start=False, stop=(h == N16H - 1),
                                )
                    for i in range(NTS):
                        osb = op.tile([P, 512], F32, tag="osb")
                        nc.vector.tensor_scalar_mul(
                            osb[:], pos[i][:], 1.0 / W2SCALE
                        )
                        nc.sync.dma_start(
                            out[t0 + i * P:t0 + (i + 1) * P, d * 512:(d + 1) * 512],
                            osb[:],
                        )

    nc.compile()
    return nc


def _get_nc():
    if "nc" not in _BUILD_CACHE:
        _BUILD_CACHE["nc"] = _build()
    return _BUILD_CACHE["nc"]


def _prep_inputs(x, num_tokens_per_expert, w1, w2, w3):
    """Host-side shard + layout prep: per-expert transposed bf16 operands."""
    x = np.asarray(x, dtype=np.float32)
    w1 = np.asarray(w1)
    w2 = np.asarray(w2)
    w3 = np.asarray(w3)
    counts = np.asarray(num_tokens_per_expert).astype(np.int64)
    offs = np.concatenate([[0], np.cumsum(counts)])

    in_maps = []
    for e in range(E):
        n_e = int(counts[e])
        if n_e > T:
            raise ValueError(f"expert {e} has {n_e} tokens > padded capacity {T}")
        xe = x[offs[e]:offs[e] + n_e]
        if n_e < T:
            xe = np.concatenate(
                [xe, np.zeros((T - n_e, DIM), dtype=np.float32)], axis=0
            )
        w2t = np.ascontiguousarray(np.asarray(w2[e]).T) * W2SCALE
        w2t8 = np.clip(w2t[: N8H * P], -240.0, 240.0).astype(
            ml_dtypes.float8_e4m3
        )
        w2t16 = w2t[N8H * P:].astype(ml_dtypes.bfloat16)
        in_maps.append({
            "xt": np.ascontiguousarray(xe.T).astype(ml_dtypes.bfloat16),
            "w1t": np.ascontiguousarray(np.asarray(w1[e]).T).astype(ml_dtypes.bfloat16),
            "w3t": np.ascontiguousarray(np.asarray(w3[e]).T).astype(ml_dtypes.bfloat16),
            "w2t8": w2t8,
            "w2t16": np.ascontiguousarray(w2t16),
        })
    return in_maps, counts


def _run(inputs, **run_kwargs):
    in_maps, counts = _prep_inputs(
        inputs["x"], inputs["num_tokens_per_expert"],
        inputs["w1"], inputs["w2"], inputs["w3"],
    )
    nc = _get_nc()
    res = run_bass_kernel_spmd(nc, in_maps, core_ids=list(range(E)), **run_kwargs)
    pieces = [res.results[e]["out"][: int(counts[e])] for e in range(E)]
    full = np.concatenate(pieces, axis=0).astype(np.float32)
    return full, res


def kernel(**inputs):
    out, _ = _run(inputs)
    return out


if __name__ == "__main__":
    # Tiny self-check with random data (not the reference inputs).
    rng = np.random.default_rng(0)
    ins = {
        "x": rng.standard_normal((E * T, DIM), dtype=np.float32),
        "num_tokens_per_expert": np.full((E,), T, dtype=np.int64),
        "w1": rng.standard_normal((E, HID, DIM), dtype=np.float32) * 0.02,
        "w2": rng.standard_normal((E, DIM, HID), dtype=np.float32) * 0.02,
        "w3": rng.standard_normal((E, HID, DIM), dtype=np.float32) * 0.02,
    }
    got = kernel(**ins)
    print("out shape:", got.shape, got.dtype)



# revision 11
# speedup vs baseline: 1.2424x; 1.0015x over previous
"""FP8GroupedExperts Trainium2 kernel (expert-parallel over 8 NeuronCores).

Math per expert e (tokens pre-sorted by expert, n_e tokens each):
    h   = silu(x_e @ w1[e].T) * (x_e @ w3[e].T)      # (n_e, HID) SwiGLU
    out = h @ w2[e].T                                 # (n_e, DIM)

Sharding: one expert per core (E == n_cores == 8). Each core's segment is
padded to T tokens; zero rows produce zero outputs (silu(0)*0 == 0) and are
stripped on the host after the gather.

On-chip layout (zero on-chip transposes):
  phase 1 computes h^T tiles (HID on partitions, tokens on free dim):
      y1T = W1 @ x^T  via  matmul(lhsT=W1T chunk, rhs=xT chunk)
      hT  = silu(y1T) * y3T
  phase 2 computes out = h @ W2^T with the HID contraction on partitions:
      out tile = matmul(lhsT=hT chunk, rhs=W2T chunk)
All operands are host-pre-transposed so every DMA is dense.
"""

import sys

for _p in ("/opt/trn_rl_repo",):
    if _p not in sys.path:
        sys.path.append(_p)

import numpy as np
import ml_dtypes

import concourse.bacc as bacc
import concourse.mybir as mybir
import concourse.tile as tile
from concourse.bass import ts
from concourse.bass_utils import run_bass_kernel_spmd

E = 8
DIM = 2048
HID = 5632
T = 2048          # padded tokens per expert (= TOT // E)
P = 128

T_TILE = 512
NT = T // T_TILE            # 4 token tiles
KSUB = DIM // P             # 16 contraction subtiles for phase 1
HSUB = HID // P             # 44 h subtiles (phase-1 outputs / phase-2 contraction)
HG = 4                      # h-subtiles per weight-group DMA
NHG = HSUB // HG            # 11 weight groups
ND = DIM // 512             # 4 output column chunks (phase 2)
NTS = T_TILE // P           # 4 token partition-subtiles (phase 2)

# Partial-fp8 phase 2: the first N8H h-subtiles of the HID contraction run as
# e4m3 DoubleRow pairs (2 subtiles per matmul at ~2x rate). h is quantized at
# scale 1 (values ~N(0,0.5), well inside e4m3 range); w2 is pre-scaled by 2^10
# on the host for BOTH the fp8 and bf16 parts (a lossless exponent shift for
# bf16), so the PSUM accumulates out*2^10 uniformly and one scaled copy
# descales it. Error budget: full-fp8 phase 2 measures 3.75% rel err vs the
# fp32 oracle; N8H/44 of the contraction in fp8 gives 3.75%*sqrt(N8H/44),
# +0.4% bf16 baseline in quadrature -> ~1.8% at N8H=10 vs the 2e-2 gate.
N8H = 10                    # fp8 h-subtiles (must be even)
N16H = HSUB - N8H           # remaining bf16 h-subtiles
W2SCALE = 1024.0            # host-side w2 pre-scale (power of 2)

BF16 = mybir.dt.bfloat16
F8E4 = mybir.dt.float8e4
F32 = mybir.dt.float32

_BUILD_CACHE = {}


def _build():
    """Build the per-core Bass module (same NEFF for all 8 cores)."""
    nc = bacc.Bacc(None, target_bir_lowering=False)

    xt = nc.dram_tensor("xt", [DIM, T], BF16, kind="ExternalInput")
    w1t = nc.dram_tensor("w1t", [DIM, HID], BF16, kind="ExternalInput")
    w3t = nc.dram_tensor("w3t", [DIM, HID], BF16, kind="ExternalInput")
    w2t8 = nc.dram_tensor("w2t8", [N8H * P, DIM], F8E4, kind="ExternalInput")
    w2t16 = nc.dram_tensor("w2t16", [N16H * P, DIM], BF16, kind="ExternalInput")
    out = nc.dram_tensor("out", [T, DIM], F32, kind="ExternalOutput")

    xt_r = xt.rearrange("(ko p) t -> p ko t", p=P)      # [128, 16, 2048]
    w1_r = w1t.rearrange("(ko p) h -> p ko h", p=P)     # [128, 16, 5632]
    w3_r = w3t.rearrange("(ko p) h -> p ko h", p=P)

    w2_r8 = w2t8.rearrange("(hh p) d -> p hh d", p=P)    # [128, 10, 2048]
    w2_r16 = w2t16.rearrange("(hh p) d -> p hh d", p=P)  # [128, 34, 2048]

    with tile.TileContext(nc) as tc:
        with (
            tc.tile_pool(name="xp", bufs=2) as xp,
            tc.tile_pool(name="wp", bufs=2) as wp,
            tc.tile_pool(name="hp", bufs=1) as hp,
            tc.tile_pool(name="tp", bufs=3) as tp,
            tc.tile_pool(name="w2p", bufs=5) as w2p,
            tc.tile_pool(name="op", bufs=4) as op,
            tc.tile_pool(name="ps1", bufs=1, space="PSUM") as ps1,
            tc.tile_pool(name="ps2", bufs=6, space="PSUM") as ps2,
        ):
            def load_x(it, split=1):
                # split>1 issues per-k-chunk DMAs so the first matmuls (which
                # only read low k subtiles) can start before the full tile lands
                t0 = it * T_TILE
                xtt = xp.tile([P, KSUB, T_TILE], BF16, tag="xtt", name=f"xtt_{it}")
                kc = KSUB // split
                for s in range(split):
                    nc.sync.dma_start(
                        xtt[:, s * kc:(s + 1) * kc, :],
                        xt_r[:, s * kc:(s + 1) * kc, t0:t0 + T_TILE],
                    )
                return xtt

            def load_wg(it, hg, split=1):
                h0 = hg * HG * P
                w1g = wp.tile(
                    [P, KSUB, HG * P], BF16, tag="w1g", name=f"w1g_{it}_{hg}"
                )
                w3g = wp.tile(
                    [P, KSUB, HG * P], BF16, tag="w3g", name=f"w3g_{it}_{hg}"
                )
                kc = KSUB // split
                for s in range(split):
                    ks = slice(s * kc, (s + 1) * kc)
                    nc.sync.dma_start(w1g[:, ks, :], w1_r[:, ks, h0:h0 + HG * P])
                    nc.sync.dma_start(w3g[:, ks, :], w3_r[:, ks, h0:h0 + HG * P])
                return w1g, w3g

            pending = {}

            # HAM pre-warm: the PE clock gate defaults to 1.2GHz and opens to
            # 2.4GHz only after ~3.4us of sustained activity. Run zero-operand
            # matmuls into a scratch PSUM bank while the first input DMAs are
            # in flight, so real matmuls start at full clock. The scratch tile
            # shares the py1 slot; PE program order keeps the reuse safe.
            wz = tp.tile([P, 512], BF16, tag="wz", bufs=1)
            nc.gpsimd.memset(wz[:], 0.0)
            wps = ps1.tile([P, T_TILE], F32, tag="py1", name="warm_ps")
            for _ in range(28):
                nc.tensor.matmul(wps[:], wz[:, 0:P], wz[:], start=True, stop=True)

            # iter-0 startup: interleave small per-k chunks of x/w1 first (the
            # py1 k-loop consumes those), then w3 (py3 runs a full k-loop
            # later), so the first matmuls' operands land as early as possible
            # (DMA issue on the sync queue serializes)
            xtt0 = xp.tile([P, KSUB, T_TILE], BF16, tag="xtt", name="xtt_0")
            w1g0 = wp.tile([P, KSUB, HG * P], BF16, tag="w1g", name="w1g_0_0")
            w3g0 = wp.tile([P, KSUB, HG * P], BF16, tag="w3g", name="w3g_0_0")
            for s in range(4):
                ks = slice(s * (KSUB // 4), (s + 1) * (KSUB // 4))
                nc.sync.dma_start(xtt0[:, ks, :], xt_r[:, ks, 0:T_TILE])
                nc.sync.dma_start(w1g0[:, ks, :], w1_r[:, ks, 0:HG * P])
            for s in range(4):
                ks = slice(s * (KSUB // 4), (s + 1) * (KSUB // 4))
                nc.sync.dma_start(w3g0[:, ks, :], w3_r[:, ks, 0:HG * P])
            pending[("x", 0)] = xtt0
            pending[("w", 0, 0)] = (w1g0, w3g0)

            for it in range(NT):
                t0 = it * T_TILE
                xtt = pending.pop(("x", it), None)
                if xtt is None:
                    xtt = load_x(it)
                hsb8 = hp.tile([P, N8H, T_TILE], F8E4, tag="hsb8")
                hsb16 = hp.tile([P, N16H, T_TILE], BF16, tag="hsb16")

                # ---- phase 1: hT = silu(W1 xT) * (W3 xT), HID on partitions
                for hg in range(NHG):
                    wg = pending.pop(("w", it, hg), None)
                    if wg is None:
                        wg = load_wg(it, hg)
                    w1g, w3g = wg
                    for hh in range(HG):
                        h = hg * HG + hh
                        py1 = ps1.tile([P, T_TILE], F32, tag="py1")
                        for k in range(KSUB):
                            nc.tensor.matmul(
                                py1[:], w1g[:, k, ts(hh, P)], xtt[:, k, :],
                                start=(k == 0), stop=(k == KSUB - 1),
                            )
                        py3 = ps1.tile([P, T_TILE], F32, tag="py3")
                        for k in range(KSUB):
                            nc.tensor.matmul(
                                py3[:], w3g[:, k, ts(hh, P)], xtt[:, k, :],
                                start=(k == 0), stop=(k == KSUB - 1),
                            )
                        smp = tp.tile([P, T_TILE], F32, tag="smp")
                        nc.scalar.activation(
                            smp[:], py1[:], mybir.ActivationFunctionType.Silu
                        )
                        hdst = (
                            hsb8[:, h, :] if h < N8H else hsb16[:, h - N8H, :]
                        )
                        nc.vector.tensor_tensor(
                            hdst, smp[:], py3[:], mybir.AluOpType.mult
                        )

                # head-start phase 2's first w2 tiles (the d=0 fp8 group and
                # first bf16 group), then prefetch the next iter's activations
                # + first weight group: the head tiles aren't queued behind the
                # 6MB prefetch, and the prefetch still issues before the rest
                # of the w2 stream
                W2B = 4  # bf16 h-subtiles per w2 DMA
                w2g8_head = w2p.tile([P, N8H, 512], F8E4, tag="w2g8",
                                     name=f"w2head8_{it}")
                nc.sync.dma_start(w2g8_head[:], w2_r8[:, :, 0:512])
                w2g16_head = w2p.tile([P, W2B, 512], BF16, tag="w2g",
                                      name=f"w2head_{it}")
                nc.sync.dma_start(w2g16_head[:], w2_r16[:, 0:W2B, 0:512])

                # ---- phase 2: out tile = hT.T @ W2T, contraction over HID;
                # fp8 DoubleRow pairs first, then the bf16 remainder
                bf16_bases = list(range(0, N16H, W2B))
                for d in range(ND):
                    pos = [
                        ps2.tile([P, 512], F32, tag="po", name=f"po_{i}")
                        for i in range(NTS)
                    ]
                    if d == 0:
                        w2g8 = w2g8_head
                    else:
                        w2g8 = w2p.tile([P, N8H, 512], F8E4, tag="w2g8")
                        nc.sync.dma_start(
                            w2g8[:], w2_r8[:, :, d * 512:(d + 1) * 512]
                        )
                    for j in range(N8H // 2):
                        for i in range(NTS):
                            nc.tensor.matmul(
                                pos[i][:],
                                hsb8[:, 2 * j:2 * j + 2, ts(i, P)],
                                w2g8[:, 2 * j:2 * j + 2, :],
                                start=(j == 0), stop=False,
                                perf_mode=mybir.MatmulPerfMode.DoubleRow,
                            )
                    for hb in bf16_bases:
                        nb = min(W2B, N16H - hb)
                        if d == 0 and hb == 0:
                            w2g = w2g16_head
                        else:
                            w2g = w2p.tile([P, nb, 512], BF16, tag="w2g")
                            nc.sync.dma_start(
                                w2g[:],
                                w2_r16[:, hb:hb + nb, d * 512:(d + 1) * 512],
                            )
                        for hh in range(nb):
                            h = hb + hh
                            for i in range(NTS):
                                nc.tensor.matmul(
                                    pos[i][:], hsb16[:, h, ts(i, P)],
                                    w2g[:, hh, :],
                                    start=False, stop=(h == N16H - 1),
                                )
                    for i in range(NTS):
                        osb = op.tile([P, 512], F32, tag="osb")
                        nc.vector.tensor_scalar_mul(
                            osb[:], pos[i][:], 1.0 / W2SCALE
                        )
                        nc.sync.dma_start(
                            out[t0 + i * P:t0 + (i + 1) * P, d * 512:(d + 1) * 512],
                            osb[:],
                        )
                    # next-iter prefetch issues after d=0's w2 stream so the
                    # 6MB of x/w1/w3 doesn't delay this iter's w2 groups
                    if d == 0 and it + 1 < NT:
                        pending[("x", it + 1)] = load_x(it + 1)
                        pending[("w", it + 1, 0)] = load_wg(it + 1, 0)

    nc.compile()
    return nc


def _get_nc():
    if "nc" not in _BUILD_CACHE:
        _BUILD_CACHE["nc"] = _build()
    return _BUILD_CACHE["nc"]


def _prep_inputs(x, num_tokens_per_expert, w1, w2, w3):
    """Host-side shard + layout prep: per-expert transposed bf16 operands."""
    x = np.asarray(x, dtype=np.float32)
    w1 = np.asarray(w1)
    w2 = np.asarray(w2)
    w3 = np.asarray(w3)
    counts = np.asarray(num_tokens_per_expert).astype(np.int64)
    offs = np.concatenate([[0], np.cumsum(counts)])

    in_maps = []
    for e in range(E):
        n_e = int(counts[e])
        if n_e > T:
            raise ValueError(f"expert {e} has {n_e} tokens > padded capacity {T}")
        xe = x[offs[e]:offs[e] + n_e]
        if n_e < T:
            xe = np.concatenate(
                [xe, np.zeros((T - n_e, DIM), dtype=np.float32)], axis=0
            )
        w2t = np.ascontiguousarray(np.asarray(w2[e]).T) * W2SCALE
        w2t8 = np.clip(w2t[: N8H * P], -240.0, 240.0).astype(
            ml_dtypes.float8_e4m3
        )
        w2t16 = w2t[N8H * P:].astype(ml_dtypes.bfloat16)
        in_maps.append({
            "xt": np.ascontiguousarray(xe.T).astype(ml_dtypes.bfloat16),
            "w1t": np.ascontiguousarray(np.asarray(w1[e]).T).astype(ml_dtypes.bfloat16),
            "w3t": np.ascontiguousarray(np.asarray(w3[e]).T).astype(ml_dtypes.bfloat16),
            "w2t8": w2t8,
            "w2t16": np.ascontiguousarray(w2t16),
        })
    return in_maps, counts


def _run(inputs, **run_kwargs):
    in_maps, counts = _prep_inputs(
        inputs["x"], inputs["num_tokens_per_expert"],
        inputs["w1"], inputs["w2"], inputs["w3"],
    )
    nc = _get_nc()
    res = run_bass_kernel_spmd(nc, in_maps, core_ids=list(range(E)), **run_kwargs)
    pieces = [res.results[e]["out"][: int(counts[e])] for e in range(E)]
    full = np.concatenate(pieces, axis=0).astype(np.float32)
    return full, res


def kernel(**inputs):
    out, _ = _run(inputs)
    return out


if __name__ == "__main__":
    # Tiny self-check with random data (not the reference inputs).
    rng = np.random.default_rng(0)
    ins = {
        "x": rng.standard_normal((E * T, DIM), dtype=np.float32),
        "num_tokens_per_expert": np.full((E,), T, dtype=np.int64),
        "w1": rng.standard_normal((E, HID, DIM), dtype=np.float32) * 0.02,
        "w2": rng.standard_normal((E, DIM, HID), dtype=np.float32) * 0.02,
        "w3": rng.standard_normal((E, HID, DIM), dtype=np.float32) * 0.02,
    }
    got = kernel(**ins)
    print("out shape:", got.shape, got.dtype)



# revision 12
# speedup vs baseline: 1.2526x; 1.0082x over previous
"""FP8GroupedExperts Trainium2 kernel (expert-parallel over 8 NeuronCores).

Math per expert e (tokens pre-sorted by expert, n_e tokens each):
    h   = silu(x_e @ w1[e].T) * (x_e @ w3[e].T)      # (n_e, HID) SwiGLU
    out = h @ w2[e].T                                 # (n_e, DIM)

Sharding: one expert per core (E == n_cores == 8). Each core's segment is
padded to T tokens; zero rows produce zero outputs (silu(0)*0 == 0) and are
stripped on the host after the gather.

On-chip layout (zero on-chip transposes):
  phase 1 computes h^T tiles (HID on partitions, tokens on free dim):
      y1T = W1 @ x^T  via  matmul(lhsT=W1T chunk, rhs=xT chunk)
      hT  = silu(y1T) * y3T
  phase 2 computes out = h @ W2^T with the HID contraction on partitions:
      out tile = matmul(lhsT=hT chunk, rhs=W2T chunk)
All operands are host-pre-transposed so every DMA is dense.
"""

import sys

for _p in ("/opt/trn_rl_repo",):
    if _p not in sys.path:
        sys.path.append(_p)

import numpy as np
import ml_dtypes

import concourse.bacc as bacc
import concourse.mybir as mybir
import concourse.tile as tile
from concourse.bass import ts
from concourse.bass_utils import run_bass_kernel_spmd

E = 8
DIM = 2048
HID = 5632
T = 2048          # padded tokens per expert (= TOT // E)
P = 128

T_TILE = 512
NT = T // T_TILE            # 4 token tiles
KSUB = DIM // P             # 16 contraction subtiles for phase 1
HSUB = HID // P             # 44 h subtiles (phase-1 outputs / phase-2 contraction)
HG = 4                      # h-subtiles per weight-group DMA
NHG = HSUB // HG            # 11 weight groups
ND = DIM // 512             # 4 output column chunks (phase 2)
NTS = T_TILE // P           # 4 token partition-subtiles (phase 2)

# Partial-fp8 phase 2: the first N8H h-subtiles of the HID contraction run as
# e4m3 DoubleRow pairs (2 subtiles per matmul at ~2x rate). h is quantized at
# scale 1 (values ~N(0,0.5), well inside e4m3 range); w2 is pre-scaled by 2^10
# on the host for BOTH the fp8 and bf16 parts (a lossless exponent shift for
# bf16), so the PSUM accumulates out*2^10 uniformly and one scaled copy
# descales it. Error budget: full-fp8 phase 2 measures 3.75% rel err vs the
# fp32 oracle; N8H/44 of the contraction in fp8 gives 3.75%*sqrt(N8H/44),
# +0.4% bf16 baseline in quadrature -> ~1.8% at N8H=10 vs the 2e-2 gate.
N8H = 12                    # fp8 h-subtiles (must be even)
N16H = HSUB - N8H           # remaining bf16 h-subtiles
W2SCALE = 1024.0            # host-side w2 pre-scale (power of 2)

BF16 = mybir.dt.bfloat16
F8E4 = mybir.dt.float8e4
F32 = mybir.dt.float32

_BUILD_CACHE = {}


def _build():
    """Build the per-core Bass module (same NEFF for all 8 cores)."""
    nc = bacc.Bacc(None, target_bir_lowering=False)

    xt = nc.dram_tensor("xt", [DIM, T], BF16, kind="ExternalInput")
    w1t = nc.dram_tensor("w1t", [DIM, HID], BF16, kind="ExternalInput")
    w3t = nc.dram_tensor("w3t", [DIM, HID], BF16, kind="ExternalInput")
    w2t8 = nc.dram_tensor("w2t8", [N8H * P, DIM], F8E4, kind="ExternalInput")
    w2t16 = nc.dram_tensor("w2t16", [N16H * P, DIM], BF16, kind="ExternalInput")
    out = nc.dram_tensor("out", [T, DIM], F32, kind="ExternalOutput")

    xt_r = xt.rearrange("(ko p) t -> p ko t", p=P)      # [128, 16, 2048]
    w1_r = w1t.rearrange("(ko p) h -> p ko h", p=P)     # [128, 16, 5632]
    w3_r = w3t.rearrange("(ko p) h -> p ko h", p=P)

    w2_r8 = w2t8.rearrange("(hh p) d -> p hh d", p=P)    # [128, 10, 2048]
    w2_r16 = w2t16.rearrange("(hh p) d -> p hh d", p=P)  # [128, 34, 2048]

    with tile.TileContext(nc) as tc:
        with (
            tc.tile_pool(name="xp", bufs=2) as xp,
            tc.tile_pool(name="wp", bufs=2) as wp,
            tc.tile_pool(name="hp", bufs=1) as hp,
            tc.tile_pool(name="tp", bufs=3) as tp,
            tc.tile_pool(name="w2p", bufs=5) as w2p,
            tc.tile_pool(name="op", bufs=4) as op,
            tc.tile_pool(name="ps1", bufs=1, space="PSUM") as ps1,
            tc.tile_pool(name="ps2", bufs=6, space="PSUM") as ps2,
        ):
            def load_x(it, split=1):
                # split>1 issues per-k-chunk DMAs so the first matmuls (which
                # only read low k subtiles) can start before the full tile lands
                t0 = it * T_TILE
                xtt = xp.tile([P, KSUB, T_TILE], BF16, tag="xtt", name=f"xtt_{it}")
                kc = KSUB // split
                for s in range(split):
                    nc.sync.dma_start(
                        xtt[:, s * kc:(s + 1) * kc, :],
                        xt_r[:, s * kc:(s + 1) * kc, t0:t0 + T_TILE],
                    )
                return xtt

            def load_wg(it, hg, split=1):
                h0 = hg * HG * P
                w1g = wp.tile(
                    [P, KSUB, HG * P], BF16, tag="w1g", name=f"w1g_{it}_{hg}"
                )
                w3g = wp.tile(
                    [P, KSUB, HG * P], BF16, tag="w3g", name=f"w3g_{it}_{hg}"
                )
                kc = KSUB // split
                for s in range(split):
                    ks = slice(s * kc, (s + 1) * kc)
                    nc.sync.dma_start(w1g[:, ks, :], w1_r[:, ks, h0:h0 + HG * P])
                    nc.sync.dma_start(w3g[:, ks, :], w3_r[:, ks, h0:h0 + HG * P])
                return w1g, w3g

            pending = {}

            # HAM pre-warm: the PE clock gate defaults to 1.2GHz and opens to
            # 2.4GHz only after ~3.4us of sustained activity. Run zero-operand
            # matmuls into a scratch PSUM bank while the first input DMAs are
            # in flight, so real matmuls start at full clock. The scratch tile
            # shares the py1 slot; PE program order keeps the reuse safe.
            wz = tp.tile([P, 512], BF16, tag="wz", bufs=1)
            nc.gpsimd.memset(wz[:], 0.0)
            wps = ps1.tile([P, T_TILE], F32, tag="py1", name="warm_ps")
            for _ in range(28):
                nc.tensor.matmul(wps[:], wz[:, 0:P], wz[:], start=True, stop=True)

            # iter-0 startup: interleave small per-k chunks of x/w1 first (the
            # py1 k-loop consumes those), then w3 (py3 runs a full k-loop
            # later), so the first matmuls' operands land as early as possible
            # (DMA issue on the sync queue serializes)
            xtt0 = xp.tile([P, KSUB, T_TILE], BF16, tag="xtt", name="xtt_0")
            w1g0 = wp.tile([P, KSUB, HG * P], BF16, tag="w1g", name="w1g_0_0")
            w3g0 = wp.tile([P, KSUB, HG * P], BF16, tag="w3g", name="w3g_0_0")
            for s in range(4):
                ks = slice(s * (KSUB // 4), (s + 1) * (KSUB // 4))
                nc.sync.dma_start(xtt0[:, ks, :], xt_r[:, ks, 0:T_TILE])
                nc.sync.dma_start(w1g0[:, ks, :], w1_r[:, ks, 0:HG * P])
            for s in range(4):
                ks = slice(s * (KSUB // 4), (s + 1) * (KSUB // 4))
                nc.sync.dma_start(w3g0[:, ks, :], w3_r[:, ks, 0:HG * P])
            pending[("x", 0)] = xtt0
            pending[("w", 0, 0)] = (w1g0, w3g0)

            for it in range(NT):
                t0 = it * T_TILE
                xtt = pending.pop(("x", it), None)
                if xtt is None:
                    xtt = load_x(it)
                hsb8 = hp.tile([P, N8H, T_TILE], F8E4, tag="hsb8")
                hsb16 = hp.tile([P, N16H, T_TILE], BF16, tag="hsb16")

                # ---- phase 1: hT = silu(W1 xT) * (W3 xT), HID on partitions
                for hg in range(NHG):
                    wg = pending.pop(("w", it, hg), None)
                    if wg is None:
                        wg = load_wg(it, hg)
                    w1g, w3g = wg
                    for hh in range(HG):
                        h = hg * HG + hh
                        py1 = ps1.tile([P, T_TILE], F32, tag="py1")
                        for k in range(KSUB):
                            nc.tensor.matmul(
                                py1[:], w1g[:, k, ts(hh, P)], xtt[:, k, :],
                                start=(k == 0), stop=(k == KSUB - 1),
                            )
                        py3 = ps1.tile([P, T_TILE], F32, tag="py3")
                        for k in range(KSUB):
                            nc.tensor.matmul(
                                py3[:], w3g[:, k, ts(hh, P)], xtt[:, k, :],
                                start=(k == 0), stop=(k == KSUB - 1),
                            )
                        smp = tp.tile([P, T_TILE], F32, tag="smp")
                        nc.scalar.activation(
                            smp[:], py1[:], mybir.ActivationFunctionType.Silu
                        )
                        hdst = (
                            hsb8[:, h, :] if h < N8H else hsb16[:, h - N8H, :]
                        )
                        nc.vector.tensor_tensor(
                            hdst, smp[:], py3[:], mybir.AluOpType.mult
                        )

                # head-start phase 2's first w2 tiles (the d=0 fp8 group and
                # first bf16 group), then prefetch the next iter's activations
                # + first weight group: the head tiles aren't queued behind the
                # 6MB prefetch, and the prefetch still issues before the rest
                # of the w2 stream
                W2B = 4  # bf16 h-subtiles per w2 DMA
                w2g8_head = w2p.tile([P, N8H, 512], F8E4, tag="w2g8",
                                     name=f"w2head8_{it}")
                nc.sync.dma_start(w2g8_head[:], w2_r8[:, :, 0:512])
                w2g16_head = w2p.tile([P, W2B, 512], BF16, tag="w2g",
                                      name=f"w2head_{it}")
                nc.sync.dma_start(w2g16_head[:], w2_r16[:, 0:W2B, 0:512])

                # ---- phase 2: out tile = hT.T @ W2T, contraction over HID;
                # fp8 DoubleRow pairs first, then the bf16 remainder
                bf16_bases = list(range(0, N16H, W2B))
                for d in range(ND):
                    pos = [
                        ps2.tile([P, 512], F32, tag="po", name=f"po_{i}")
                        for i in range(NTS)
                    ]
                    if d == 0:
                        w2g8 = w2g8_head
                    else:
                        w2g8 = w2p.tile([P, N8H, 512], F8E4, tag="w2g8")
                        nc.sync.dma_start(
                            w2g8[:], w2_r8[:, :, d * 512:(d + 1) * 512]
                        )
                    for j in range(N8H // 2):
                        for i in range(NTS):
                            nc.tensor.matmul(
                                pos[i][:],
                                hsb8[:, 2 * j:2 * j + 2, ts(i, P)],
                                w2g8[:, 2 * j:2 * j + 2, :],
                                start=(j == 0), stop=False,
                                perf_mode=mybir.MatmulPerfMode.DoubleRow,
                            )
                    for hb in bf16_bases:
                        nb = min(W2B, N16H - hb)
                        if d == 0 and hb == 0:
                            w2g = w2g16_head
                        else:
                            w2g = w2p.tile([P, nb, 512], BF16, tag="w2g")
                            nc.sync.dma_start(
                                w2g[:],
                                w2_r16[:, hb:hb + nb, d * 512:(d + 1) * 512],
                            )
                        for hh in range(nb):
                            h = hb + hh
                            for i in range(NTS):
                                nc.tensor.matmul(
                                    pos[i][:], hsb16[:, h, ts(i, P)],
                                    w2g[:, hh, :],
                                    start=False, stop=(h == N16H - 1),
                                )
                    for i in range(NTS):
                        osb = op.tile([P, 512], F32, tag="osb")
                        nc.vector.tensor_scalar_mul(
                            osb[:], pos[i][:], 1.0 / W2SCALE
                        )
                        nc.sync.dma_start(
                            out[t0 + i * P:t0 + (i + 1) * P, d * 512:(d + 1) * 512],
                            osb[:],
                        )
                    # next-iter prefetch issues after d=0's w2 stream so the
                    # 6MB of x/w1/w3 doesn't delay this iter's w2 groups
                    if d == 0 and it + 1 < NT:
                        pending[("x", it + 1)] = load_x(it + 1)
                        pending[("w", it + 1, 0)] = load_wg(it + 1, 0)

    nc.compile()
    return nc


def _get_nc():
    if "nc" not in _BUILD_CACHE:
        _BUILD_CACHE["nc"] = _build()
    return _BUILD_CACHE["nc"]


def _prep_inputs(x, num_tokens_per_expert, w1, w2, w3):
    """Host-side shard + layout prep: per-expert transposed bf16 operands."""
    x = np.asarray(x, dtype=np.float32)
    w1 = np.asarray(w1)
    w2 = np.asarray(w2)
    w3 = np.asarray(w3)
    counts = np.asarray(num_tokens_per_expert).astype(np.int64)
    offs = np.concatenate([[0], np.cumsum(counts)])

    in_maps = []
    for e in range(E):
        n_e = int(counts[e])
        if n_e > T:
            raise ValueError(f"expert {e} has {n_e} tokens > padded capacity {T}")
        xe = x[offs[e]:offs[e] + n_e]
        if n_e < T:
            xe = np.concatenate(
                [xe, np.zeros((T - n_e, DIM), dtype=np.float32)], axis=0
            )
        w2t = np.ascontiguousarray(np.asarray(w2[e]).T) * W2SCALE
        w2t8 = np.clip(w2t[: N8H * P], -240.0, 240.0).astype(
            ml_dtypes.float8_e4m3
        )
        w2t16 = w2t[N8H * P:].astype(ml_dtypes.bfloat16)
        in_maps.append({
            "xt": np.ascontiguousarray(xe.T).astype(ml_dtypes.bfloat16),
            "w1t": np.ascontiguousarray(np.asarray(w1[e]).T).astype(ml_dtypes.bfloat16),
            "w3t": np.ascontiguousarray(np.asarray(w3[e]).T).astype(ml_dtypes.bfloat16),
            "w2t8": w2t8,
            "w2t16": np.ascontiguousarray(w2t16),
        })
    return in_maps, counts


def _run(inputs, **run_kwargs):
    in_maps, counts = _prep_inputs(
        inputs["x"], inputs["num_tokens_per_expert"],
        inputs["w1"], inputs["w2"], inputs["w3"],
    )
    nc = _get_nc()
    res = run_bass_kernel_spmd(nc, in_maps, core_ids=list(range(E)), **run_kwargs)
    pieces = [res.results[e]["out"][: int(counts[e])] for e in range(E)]
    full = np.concatenate(pieces, axis=0).astype(np.float32)
    return full, res


def kernel(**inputs):
    out, _ = _run(inputs)
    return out


if __name__ == "__main__":
    # Tiny self-check with random data (not the reference inputs).
    rng = np.random.default_rng(0)
    ins = {
        "x": rng.standard_normal((E * T, DIM), dtype=np.float32),
        "num_tokens_per_expert": np.full((E,), T, dtype=np.int64),
        "w1": rng.standard_normal((E, HID, DIM), dtype=np.float32) * 0.02,
        "w2": rng.standard_normal((E, DIM, HID), dtype=np.float32) * 0.02,
        "w3": rng.standard_normal((E, HID, DIM), dtype=np.float32) * 0.02,
    }
    got = kernel(**ins)
    print("out shape:", got.shape, got.dtype)

